# revision 1
# baseline (speedup 1.0000x reference)
"""Trainium2 Bass kernel for EnhancedTrajectoryPredictor GNN message passing.

Data-parallel over batch: core c handles batch element c (T=4 windows each).
v2 design: pairwise tensors live in (i-part, (j,h)-free) layout so the
attention-weighted message sum is a DVE multiply + strided abs-reduce
(|W.Z| = W|Z| since W=exp>0) instead of 128 per-i PE matmuls. The softmax
row-sum==1 identity turns the O(N^2 H^2) message matmul into S @ (Wm2@Wu1b)
folded host-side. relu(z)=(z+|z|)/2 splits the message sum into linear terms
(one W@Bm matmul) plus the abs term; the 1/2 folds into Wm1/bm1. Attention
logits use the same trick with |wa2|/2 sign-permuted into Wa1 (i-linear part
cancels in softmax, j-linear part rides the exp bias with the mask).
All four prep products come from ONE matmul (512-wide packed rhs). The four
windows are round-robin software-pipelined so PE pairwise production of
window k+1 overlaps DVE consumption of window k.
"""
import numpy as np
import ml_dtypes

import concourse.bass as bass
import concourse.mybir as mybir
import concourse.tile as tile
from concourse import bacc
from concourse.bass_types import AP

F32 = mybir.dt.float32
BF16 = mybir.dt.bfloat16

B, N, T, IN = 8, 128, 4, 45
F, H, L = 64, 128, 3
NCORES = 8
CHJ = 12                      # j's per PSUM chunk (1536 f32 = 3 banks)
CHUNKS = [(c * CHJ, min(CHJ, N - c * CHJ)) for c in range((N + CHJ - 1) // CHJ)]
NCH = len(CHUNKS)             # 11

_bf = lambda x: np.ascontiguousarray(x).astype(ml_dtypes.bfloat16)
_f32 = lambda x: np.ascontiguousarray(x).astype(np.float32)

_CACHE = {}


class _Packer:
    def __init__(self):
        self.off = 0
        self.items = {}

    def add(self, name, rows, cols):
        self.items[name] = (0, rows, self.off, cols)
        self.off += cols
        return self.items[name]


def _layout():
    pb = _Packer()
    pb.add("eye", 128, 128)
    pb.add("ones1", 97, 128)   # ones rows read at base partitions 0/32/64/96
    pb.add("onesK", 128, 1)
    pb.add("wp", IN, F)
    pb.add("wo", F, F)
    for l in range(L):
        pb.add(f"w4{l}", F, 4 * H)    # [0.5*Wm1b | Ba-w | 0.5*Wm1t | Aa-w]
        pb.add(f"w2u{l}", H, H)       # Wm2 @ Wu1_bot
        pb.add(f"u1t{l}", F, H)
        pb.add(f"u2{l}", H, F)
    pf = _Packer()
    pf.add("maskcol", N, 1)
    pf.add("eyef", 128, 128)
    pf.add("onesf", 1, 1)
    pf.add("bp", F, 1)
    pf.add("bo", F, 1)
    for l in range(L):
        pf.add(f"b256{l}", 128, 2 * H)  # [0.5*bm1 | ba1-w] row-replicated
        pf.add(f"bu1{l}", H, 1)         # bu1 + bm2 @ Wu1_bot
        pf.add(f"bu2{l}", F, 1)
    return pb, pf


_PB, _PF = _layout()


def _build_nc(p_split):
    """p_split[l] = number of wa2>=0 columns (h-permutation puts them first)."""
    nc = bacc.Bacc(None, target_bir_lowering=False, debug=False)

    d_xT = nc.declare_dram_parameter("xT", [T, IN, N], BF16, isOutput=False)
    d_wb = nc.declare_dram_parameter("wb", [128, _PB.off], BF16, isOutput=False)
    d_wf = nc.declare_dram_parameter("wf", [128, _PF.off], F32, isOutput=False)
    d_out = nc.declare_dram_parameter("out", [T, N, F], F32, isOutput=True)

    RELU = mybir.ActivationFunctionType.Relu
    EXP = mybir.ActivationFunctionType.Exp
    ADD = mybir.AluOpType.add
    SUB = mybir.AluOpType.subtract
    MULT = mybir.AluOpType.mult
    AX = mybir.AxisListType.X

    with tile.TileContext(nc) as tc:
        with (
            tc.tile_pool(name="wts", bufs=1) as wts,
            tc.tile_pool(name="st", bufs=2) as st,
            tc.tile_pool(name="sm", bufs=2) as sm,
            tc.tile_pool(name="fl", bufs=2) as fl,
            tc.tile_pool(name="mzp", bufs=2) as mzp,
            tc.tile_pool(name="pap", bufs=2) as pap,
            tc.tile_pool(name="pbig", bufs=2, space="PSUM") as pbig,
            tc.tile_pool(name="psm", bufs=2, space="PSUM") as psm,
        ):
            t_wb = wts.tile([128, _PB.off], BF16, tag="wb")
            nc.sync.dma_start(t_wb[:], d_wb[:])
            t_wf = wts.tile([128, _PF.off], F32, tag="wf")
            nc.sync.dma_start(t_wf[:], d_wf[:])

            def wb(name):
                r0, nr, c0, ncol = _PB.items[name]
                return t_wb[r0:r0 + nr, c0:c0 + ncol]

            def wf(name):
                r0, nr, c0, ncol = _PF.items[name]
                return t_wf[r0:r0 + nr, c0:c0 + ncol]

            a_eye = wb("eye")
            _ones_all = wb("ones1")
            a_ones_r = [_ones_all[64 * r:64 * r + 1, :] for r in range(2)]
            a_onesK = wb("onesK")
            a_eyef = wf("eyef")

            # per-window python-side state handles
            S_hTf = [None] * T
            S_hTb = [None] * T
            S_AAa = [None] * T
            S_BmBa = [None] * T
            S_bias = [None] * T
            S_Wij = [None] * T
            S_WexpT = [None] * T
            S_rec = [None] * T
            S_WBsb = [None] * T
            S_part = [None] * T

            def bcast4(ap_, off):
                # (128, [j:0 x 4],[h:1 x H]) from a (128, *) tile at col `off`
                return AP(ap_.tensor, ap_.offset + off,
                          [list(ap_.ap[0]), [0, 4], [1, H]])

            def taskA(w, l):
                p = p_split[l]
                if l == 0:
                    t_xT = sm.tile([IN, N], BF16, tag="xT")
                    nc.sync.dma_start(t_xT[:], d_xT[w])
                    p_pr = psm.tile([128, 512], F32, tag="mm")
                    nc.tensor.matmul(p_pr[:F, :N], wb("wp"), t_xT[:],
                                     start=True, stop=True)
                    hTf = st.tile([F, N], F32, tag=f"hTf{w}")
                    nc.vector.tensor_scalar(hTf[:], p_pr[:F, :N], wf("bp"),
                                            None, ADD)
                    hTb = st.tile([F, N], BF16, tag=f"hTb{w}")
                    nc.vector.tensor_copy(hTb[:], hTf[:])
                    S_hTf[w], S_hTb[w] = hTf, hTb

                hTb = S_hTb[w]
                # ---- prep: ONE matmul -> [Bm' | Ba | A' | Aa] (N, 512)
                p_prep = psm.tile([128, 512], F32, tag="mm")
                nc.tensor.matmul(p_prep[:], hTb[:], wb(f"w4{l}"),
                                 start=True, stop=True)
                BmBa = sm.tile([N, 2 * H], BF16, tag="BmBa")
                nc.vector.tensor_tensor(BmBa[:], p_prep[:, 0:2 * H],
                                        wf(f"b256{l}"), ADD)
                AAa = sm.tile([N, 2 * H], BF16, tag="AAa")
                nc.vector.tensor_copy(AAa[:], p_prep[:, 2 * H:4 * H])
                S_AAa[w], S_BmBa[w] = AAa, BmBa

                # ---- j-bias column: bc[j] = sum_h sgn*Ba[j,h]; + mask
                bsl = BmBa[:, H:2 * H]
                biascol = sm.tile([N, 1], F32, tag="biascol")
                if 0 < p < H:
                    bpos = sm.tile([N, 1], F32, tag="bpos")
                    nc.vector.tensor_reduce(bpos[:], bsl[:, 0:p], AX, ADD)
                    bneg = sm.tile([N, 1], F32, tag="bneg")
                    nc.vector.tensor_reduce(bneg[:], bsl[:, p:H], AX, ADD)
                    nc.vector.scalar_tensor_tensor(biascol[:], bpos[:],
                                                   wf("maskcol"), bneg[:],
                                                   ADD, SUB)
                else:
                    sgn0 = 1.0 if p == H else -1.0
                    bpos = sm.tile([N, 1], F32, tag="bpos")
                    nc.vector.tensor_reduce(bpos[:], bsl[:], AX, ADD)
                    bc = sm.tile([N, 1], F32, tag="bneg")
                    nc.vector.tensor_scalar(bc[:], bpos[:], sgn0, None, MULT)
                    nc.vector.tensor_tensor(biascol[:], bc[:], wf("maskcol"),
                                            ADD)
                S_bias[w] = biascol

                # ---- flatten [Bm'|Ba] to 2 rows (at partitions 0/64)
                flat2 = fl.tile([65, N * 2 * H // 2], BF16, tag="flat2")
                for k in range(4):
                    eng = nc.sync if k % 2 == 0 else nc.gpsimd
                    eng.dma_start(flat2[64 * (k // 2):64 * (k // 2) + 1,
                                        (k % 2) * 8192:(k % 2 + 1) * 8192],
                                  BmBa[k * 32:(k + 1) * 32, :])

                def flat_rhs(j0, sel):
                    r = j0 // 64
                    a = flat2[64 * r:64 * r + 1, :]
                    return AP(a.tensor, a.offset + (j0 - 64 * r) * 256 + sel * H,
                              [list(a.ap[0]), [256, 4], [1, H]])

                # ---- Za chunks -> attention logits (i, j)
                logits = sm.tile([N, N], F32, tag="logits")
                for (j0, jc) in CHUNKS:
                    nq = jc // 4
                    pz = pbig.tile([128, CHJ * H], F32, tag="chunk")
                    for q in range(nq):
                        nc.tensor.matmul(pz[:, q * 512:(q + 1) * 512], a_eye,
                                         bcast4(AAa[:], H), start=True,
                                         stop=False)
                    for q in range(nq):
                        nc.tensor.matmul(pz[:, q * 512:(q + 1) * 512],
                                         a_ones_r[(j0 + 4 * q) // 64],
                                         flat_rhs(j0 + 4 * q, 1), start=False,
                                         stop=True)
                    pa = pz[:]
                    if 0 < p < H:
                        tpos = sm.tile([128, CHJ], F32, tag="tpos")
                        nc.vector.tensor_reduce(
                            tpos[:, :jc], AP(pz.tensor, pa.offset,
                                             [list(pa.ap[0]), [H, jc], [1, p]]),
                            AX, ADD, apply_absolute_value=True)
                        tneg = sm.tile([128, CHJ], F32, tag="tneg")
                        nc.vector.tensor_reduce(
                            tneg[:, :jc], AP(pz.tensor, pa.offset + p,
                                             [list(pa.ap[0]), [H, jc],
                                              [1, H - p]]),
                            AX, ADD, apply_absolute_value=True)
                        nc.vector.tensor_tensor(logits[:, j0:j0 + jc],
                                                tpos[:, :jc], tneg[:, :jc], SUB)
                    else:
                        sgn = 1.0 if p == H else -1.0
                        tpos = sm.tile([128, CHJ], F32, tag="tpos")
                        nc.vector.tensor_reduce(
                            tpos[:, :jc], AP(pz.tensor, pa.offset,
                                             [list(pa.ap[0]), [H, jc], [1, H]]),
                            AX, ADD, apply_absolute_value=True)
                        nc.vector.tensor_scalar(logits[:, j0:j0 + jc],
                                                tpos[:, :jc], sgn, None, MULT)

                # ---- softmax pieces: transpose, exp(+bias+mask), back
                p_lt = psm.tile([128, 512], F32, tag="mm")
                nc.tensor.transpose(p_lt[:N, :N], logits[:], a_eyef)
                WexpT = sm.tile([N, N], BF16, tag="WexpT")
                nc.scalar.activation(WexpT[:], p_lt[:N, :N], EXP,
                                     bias=biascol[:], scale=1.0)
                S_WexpT[w] = WexpT
                p_wij = psm.tile([N, N], BF16, tag="mm")
                nc.tensor.transpose(p_wij[:], WexpT[:], a_eye)
                Wij = sm.tile([N, N], F32, tag="Wij")
                nc.vector.tensor_copy(Wij[:], p_wij[:])
                S_Wij[w] = Wij
                p_den = psm.tile([1, N], F32, tag="mm")
                nc.tensor.matmul(p_den[:], a_onesK, WexpT[:], start=True,
                                 stop=True)
                rec_row = sm.tile([1, N], F32, tag="rec_row")
                nc.vector.reciprocal(rec_row[:], p_den[:])
                p_rc = psm.tile([N, 1], F32, tag="mm")
                nc.tensor.matmul(p_rc[:], rec_row[:], wf("onesf"), start=True,
                                 stop=True)
                rec_col = sm.tile([N, 1], F32, tag="rec_col")
                nc.vector.tensor_copy(rec_col[:], p_rc[:])
                S_rec[w] = rec_col
                p_WB = psm.tile([N, H], F32, tag="mm")
                nc.tensor.matmul(p_WB[:], WexpT[:], BmBa[:, 0:H], start=True,
                                 stop=True)
                WBsb = sm.tile([N, H], F32, tag="WBsb")
                nc.vector.tensor_copy(WBsb[:], p_WB[:])
                S_WBsb[w] = WBsb

                # ---- Zm chunks -> weighted abs message partials
                partials = pap.tile([128, NCH * H], F32, tag="part")
                for ci, (j0, jc) in enumerate(CHUNKS):
                    nq = jc // 4
                    pm = pbig.tile([128, CHJ * H], F32, tag="chunk")
                    for q in range(nq):
                        nc.tensor.matmul(pm[:, q * 512:(q + 1) * 512], a_eye,
                                         bcast4(AAa[:], 0), start=True,
                                         stop=False)
                    for q in range(nq):
                        nc.tensor.matmul(pm[:, q * 512:(q + 1) * 512],
                                         a_ones_r[(j0 + 4 * q) // 64],
                                         flat_rhs(j0 + 4 * q, 0), start=False,
                                         stop=True)
                    mz = mzp.tile([128, CHJ * H], BF16, tag="mz")
                    pmv = pm[:]
                    mzv0 = mz[:]
                    nc.vector.tensor_tensor(
                        AP(mz.tensor, mzv0.offset,
                           [list(mzv0.ap[0]), [H, jc], [1, H]]),
                        AP(pm.tensor, pmv.offset,
                           [list(pmv.ap[0]), [H, jc], [1, H]]),
                        AP(Wij.tensor, Wij[:].offset + j0,
                           [list(Wij[:].ap[0]), [1, jc], [0, H]]),
                        MULT)
                    mzv = mz[:]
                    nc.vector.tensor_reduce(
                        partials[:, ci * H:(ci + 1) * H],
                        AP(mz.tensor, mzv.offset,
                           [list(mzv.ap[0]), [1, H], [H, jc]]),
                        AX, ADD, apply_absolute_value=True)
                S_part[w] = partials

            def taskB(w, l):
                partials, WBsb, rec_col = S_part[w], S_WBsb[w], S_rec[w]
                AAa, hTf, hTb = S_AAa[w], S_hTf[w], S_hTb[w]
                pv = partials[:]
                Tfin = sm.tile([N, H], F32, tag="Tfin")
                nc.vector.tensor_reduce(
                    Tfin[:], AP(partials.tensor, pv.offset,
                                [list(pv.ap[0]), [1, H], [H, NCH]]),
                    AX, ADD)
                t1 = sm.tile([N, H], F32, tag="t1")
                nc.vector.tensor_tensor(t1[:], WBsb[:], Tfin[:], ADD)
                Sb = sm.tile([N, H], BF16, tag="Sb")
                nc.vector.scalar_tensor_tensor(Sb[:], t1[:], rec_col[:],
                                               AAa[:, 0:H], MULT, ADD)
                p_st = psm.tile([N, H], BF16, tag="mm")
                nc.tensor.transpose(p_st[:], Sb[:], a_eye)
                ST = sm.tile([H, N], BF16, tag="ST")
                nc.vector.tensor_copy(ST[:], p_st[:])

                p_u1 = psm.tile([H, N], F32, tag="mm")
                nc.tensor.matmul(p_u1[:], wb(f"u1t{l}"), hTb[:], start=True,
                                 stop=False)
                nc.tensor.matmul(p_u1[:], wb(f"w2u{l}"), ST[:], start=False,
                                 stop=True)
                u1 = sm.tile([H, N], BF16, tag="u1")
                nc.scalar.activation(u1[:], p_u1[:], RELU, bias=wf(f"bu1{l}"))
                p_u2 = psm.tile([128, 512], F32, tag="mm")
                nc.tensor.matmul(p_u2[:F, :N], wb(f"u2{l}"), u1[:], start=True,
                                 stop=True)
                hTf_new = st.tile([F, N], F32, tag=f"hTf{w}")
                nc.vector.scalar_tensor_tensor(hTf_new[:], p_u2[:F, :N],
                                               wf(f"bu2{l}"), hTf[:], ADD, ADD)
                hTb_new = st.tile([F, N], BF16, tag=f"hTb{w}")
                nc.vector.tensor_copy(hTb_new[:], hTf_new[:])
                S_hTf[w], S_hTb[w] = hTf_new, hTb_new

                if l == L - 1:
                    p_o = psm.tile([128, 512], F32, tag="mm")
                    nc.tensor.matmul(p_o[:F, :N], wb("wo"), hTb_new[:],
                                     start=True, stop=True)
                    oT = sm.tile([F, N], F32, tag="oT")
                    nc.vector.tensor_scalar(oT[:], p_o[:F, :N], wf("bo"),
                                            None, ADD)
                    p_on = psm.tile([128, 512], F32, tag="mm")
                    nc.tensor.transpose(p_on[:N, :F], oT[:],
                                        a_eyef[:F, :F])
                    o_sb = sm.tile([N, F], F32, tag="o_sb")
                    nc.vector.tensor_copy(o_sb[:], p_on[:N, :F])
                    nc.sync.dma_start(d_out[w], o_sb[:])

            tasks = [(k % T, k // T) for k in range(T * L)]
            taskA(*tasks[0])
            for k in range(1, len(tasks)):
                taskA(*tasks[k])
                taskB(*tasks[k - 1])
            taskB(*tasks[-1])

    nc.compile()
    return nc


def _pack_blobs(core_mask, Wp, bp, Wm1, bm1, Wm2, bm2, Wa1, ba1, Wa2, ba2,
                Wu1, bu1, Wu2, bu2, Wo, bo, perms):
    wb_blob = np.zeros((128, _PB.off), np.float32)
    wf_blob = np.zeros((128, _PF.off), np.float32)

    def putb(name, v):
        r0, nr, c0, ncol = _PB.items[name]
        wb_blob[r0:r0 + nr, c0:c0 + ncol] = v

    def putf(name, v):
        r0, nr, c0, ncol = _PF.items[name]
        wf_blob[r0:r0 + nr, c0:c0 + ncol] = v

    putb("eye", np.eye(128, dtype=np.float32))
    putb("ones1", np.ones((97, 128), np.float32))
    putb("onesK", np.ones((128, 1), np.float32))
    putb("wp", Wp)
    putb("wo", Wo)
    putf("maskcol", ((core_mask - 1.0) * 3.0e38).reshape(N, 1))
    putf("eyef", np.eye(128, dtype=np.float32))
    putf("onesf", np.ones((1, 1), np.float32))
    putf("bp", bp.reshape(F, 1))
    putf("bo", bo.reshape(F, 1))
    for l in range(L):
        perm, scale = perms[l]
        aw = 0.5 * scale  # |wa2|/2, permuted order
        w4 = np.concatenate([
            0.5 * Wm1[l][F:],                       # -> Bm'
            Wa1[l][F:][:, perm] * aw[None, :],      # -> Ba (scaled)
            0.5 * Wm1[l][:F],                       # -> A'
            Wa1[l][:F][:, perm] * aw[None, :],      # -> Aa (scaled)
        ], axis=1)
        putb(f"w4{l}", w4)
        putb(f"w2u{l}", Wm2[l] @ Wu1[l][F:])
        putb(f"u1t{l}", Wu1[l][:F])
        putb(f"u2{l}", Wu2[l])
        b256 = np.concatenate([0.5 * bm1[l], ba1[l][perm] * aw])
        putf(f"b256{l}", np.broadcast_to(b256, (128, 2 * H)))
        putf(f"bu1{l}", (bu1[l] + bm2[l] @ Wu1[l][F:]).reshape(H, 1))
        putf(f"bu2{l}", bu2[l].reshape(F, 1))
    return wb_blob, wf_blob


def prepare(**inputs):
    args = {k: np.asarray(v) for k, v in inputs.items()}
    x, masks = _f32(args["x"]), _f32(args["masks"])
    Wa2 = _f32(args["Wa2"])

    # sign-split permutation per layer: wa2>=0 columns first, |wa2| folded in
    perms, p_split = [], []
    for l in range(L):
        wa2 = Wa2[l][:, 0]
        order = np.argsort(~(wa2 >= 0), kind="stable")  # positives first
        perms.append((order, np.abs(wa2)[order]))
        p_split.append(int((wa2 >= 0).sum()))

    key = tuple(p_split)
    if key not in _CACHE:
        _CACHE[key] = _build_nc(p_split)
    nc = _CACHE[key]

    wkeys = dict(Wp=args["Wp"], bp=args["bp"], Wm1=args["Wm1"], bm1=args["bm1"],
                 Wm2=args["Wm2"], bm2=args["bm2"], Wa1=args["Wa1"], ba1=args["ba1"],
                 Wa2=Wa2, ba2=args["ba2"], Wu1=args["Wu1"], bu1=args["bu1"],
                 Wu2=args["Wu2"], bu2=args["bu2"], Wo=args["Wo"], bo=args["bo"])
    in_maps = []
    for c in range(NCORES):
        wb_blob, wf_blob = _pack_blobs(masks[c], perms=perms, **wkeys)
        in_maps.append({
            "xT": _bf(np.transpose(x[c], (1, 2, 0))),
            "wb": _bf(wb_blob),
            "wf": _f32(wf_blob),
        })
    return nc, in_maps


def kernel(**inputs) -> np.ndarray:
    from concourse.bass_utils import run_bass_kernel_spmd
    nc, in_maps = prepare(**inputs)
    res = run_bass_kernel_spmd(nc, in_maps, list(range(NCORES)))
    out = np.stack([np.transpose(np.asarray(res.results[c]["out"], np.float32),
                                 (1, 0, 2)) for c in range(NCORES)])
    return out

